# revision 9
# baseline (speedup 1.0000x reference)
"""Trainium2 Bass kernel for nn_Attn additive-attention scoring.

Reference computation (B=32, S=4096, H=512):
    cat    = concat([x, broadcast(h)], -1)            # (B,S,2H)
    t      = tanh(cat @ W_w^T + W_b)                  # (B,S,H)
    v      = t @ V_w^T + V_b                          # (B,S,H)
    scores = v.sum(-1, keepdims=True)                 # (B,S,1)
    attn   = softmax(scores, axis=1)
    res    = attn * x
    return res, attn

Restructuring used here (numerically equivalent to ~1e-6):
    scores[b,s] = tanh(x[b,s] @ W1^T + bias[b]) . vcol + const
where W1 = W_w[:, :H], bias[b] = h[b] @ W_w[:, H:]^T + W_b (host, tiny),
vcol = V_w.sum(0).  The constant (V_b.sum()) drops out of the softmax.

Sharding: data-parallel over batch, 4 batches per core, weights replicated.

Device dataflow per core (one batch at a time, software-pipelined):
  phase 1   stream x[b] in 1 MiB blocks into SBUF (resident, 8 MiB/batch),
            PE-transpose each block, fp32r matmul vs W1^T, tanh (+bias) on
            ACT, then a vcol matvec on PE (pipelined one block behind so PE
            never stalls on ACT) accumulates the scores row.
  softmax   on-chip over the [1, 4096] scores row (exp in place with
            accum_out); its serial chain is emitted two blocks into the
            NEXT batch's phase 1 so the PE never waits on it.
  phase 2   multiply the *resident* x blocks by raw-exp attn (PE-transposed
            to the partition dim) and 1/Z (second tensor_scalar operand)
            in place and stream res out.
  x is read from HBM exactly once: ~64 MiB DMA per core total.
"""

import sys

sys.path.insert(0, "/opt/trn_rl_repo")

import numpy as np

B, S, H = 32, 4096, 512
N_CORES = 8
BC = B // N_CORES      # batches per core
NBLK = S // 512        # 512-row blocks per batch
NT = S // 128          # 128-row tiles per batch

TRACE = False          # unused; profiling is driven externally (test.py)
LAST_RESULTS = None    # BassKernelResults of the last run

_compiled = None


def _build():
    import concourse.bacc as bacc
    import concourse.mybir as mybir
    from concourse import tile

    f32 = mybir.dt.float32
    f32r = mybir.dt.float32r
    TANH = mybir.ActivationFunctionType.Tanh
    EXP = mybir.ActivationFunctionType.Exp
    MULT = mybir.AluOpType.mult

    nc = bacc.Bacc("TRN2", target_bir_lowering=False, debug=False,
                   num_devices=N_CORES)

    x_d = nc.dram_tensor("x", [BC * S, H], f32, kind="ExternalInput").ap()
    w1t_d = nc.dram_tensor("w1t", [128, 4 * H], f32, kind="ExternalInput").ap()
    vcol_d = nc.dram_tensor("vcol", [128, 4], f32, kind="ExternalInput").ap()
    bias_d = nc.dram_tensor("bias", [128, 4 * BC], f32, kind="ExternalInput").ap()
    iden_d = nc.dram_tensor("iden", [128, 128], f32, kind="ExternalInput").ap()
    res_d = nc.dram_tensor("res", [BC * S, H], f32, kind="ExternalOutput").ap()
    attn_d = nc.dram_tensor("attn", [BC, S], f32, kind="ExternalOutput").ap()

    with tile.TileContext(nc) as tc:
        with (
            tc.tile_pool(name="const", bufs=1) as constp,
            tc.tile_pool(name="xin", bufs=14) as xinp,
            tc.tile_pool(name="xtsb", bufs=8) as xtp,
            tc.tile_pool(name="tt", bufs=8) as ttp,
            tc.tile_pool(name="soft", bufs=2) as softp,
            tc.tile_pool(name="small", bufs=4) as smallp,
            tc.tile_pool(name="atn", bufs=2) as atnp,
            tc.tile_pool(name="ps_xt", bufs=2, space="PSUM") as ps_xt,
            tc.tile_pool(name="ps_pre", bufs=2, space="PSUM") as ps_pre,
            tc.tile_pool(name="ps_sc", bufs=2, space="PSUM") as ps_sc,
            tc.tile_pool(name="ps_at", bufs=1, space="PSUM") as ps_at,
        ):
            w1t_f = xinp.tile([128, 4 * H], f32, name="w1t_f_sb", tag="xin")
            nc.sync.dma_start(w1t_f[:, :], w1t_d[:, :])
            w1t = constp.tile([128, 4 * H], f32r, name="w1t_sb")
            nc.vector.tensor_copy(w1t[:, :], w1t_f[:, :])
            vcol_f = constp.tile([128, 4], f32, name="vcol_f_sb")
            nc.sync.dma_start(vcol_f[:, :], vcol_d[:, :])
            vcol = constp.tile([128, 4], f32r, name="vcol_sb")
            nc.vector.tensor_copy(vcol[:, :], vcol_f[:, :])
            bias = constp.tile([128, 4 * BC], f32, name="bias_sb")
            nc.sync.dma_start(bias[:, :], bias_d[:, :])
            iden = constp.tile([128, 128], f32, name="iden_sb")
            nc.sync.dma_start(iden[:, :], iden_d[:, :])
            ones = constp.tile([1, 128], f32, name="ones_sb")
            nc.vector.memset(ones[0:1, :], 1.0)

            # PE warm-up: a few fp32 matmuls on the weights while the first
            # x block is still in flight, so the HAM clock gate opens before
            # real work starts.  Output is never read.
            warm_ps = ps_sc.tile([128, 512], f32, name="warm_ps", tag="sc")
            for _w in range(4):
                nc.tensor.matmul(warm_ps[:, :], w1t_f[:, 0:128],
                                 w1t_f[:, 0:512], start=True, stop=True)

            # deferred emitters for cross-block/batch software pipelining
            pending_mv = []    # score matvec of the previous block
            pending_soft = []  # softmax + attnT of the previous batch
            pending_ph2 = []   # phase-2 of the previous batch

            def emit_ph2(b, blk, xin_t, attn_t, rec128):
                s0 = b * S + blk * 512
                for ss in range(4):
                    j = blk * 4 + ss
                    nc.vector.tensor_scalar(
                        xin_t[:, ss, :], xin_t[:, ss, :],
                        attn_t[:, j:j + 1], rec128[:, 0:1],
                        op0=MULT, op1=MULT,
                    )
                nc.scalar.dma_start(
                    res_d[s0:s0 + 512, :].rearrange("(ss p) k -> p ss k", p=128),
                    xin_t[:, :, :],
                )

            def emit_soft(b, scores, bmax, xin_tiles):
                # softmax over S for batch b (exp in place over scores)
                mx = smallp.tile([1, 1], f32, name=f"mx_{b}", tag="mx")
                nc.vector.reduce_max(mx[0:1, 0:1], bmax[0:1, :],
                                     axis=mybir.AxisListType.X, negate=True)
                sm = smallp.tile([1, 1], f32, name=f"sm_{b}", tag="sm")
                nc.scalar.activation(scores[0:1, :], scores[0:1, :], EXP,
                                     bias=mx[0:1, 0:1], scale=1.0,
                                     accum_out=sm[0:1, 0:1])
                rec = smallp.tile([1, 1], f32, name=f"rec_{b}", tag="rec")
                nc.vector.reciprocal(rec[0:1, 0:1], sm[0:1, 0:1])

                # broadcast 1/Z to all 128 partitions: ones^T @ rec on PE
                rc_ps = ps_at.tile([128, 1], f32, name=f"rcps_{b}", tag="rcb")
                nc.tensor.matmul(rc_ps[:, 0:1], ones[0:1, :], rec[0:1, 0:1],
                                 start=True, stop=True)
                rec128 = atnp.tile([128, 1], f32, name=f"rec128_{b}", tag="rc")
                nc.vector.tensor_copy(rec128[:, 0:1], rc_ps[:, 0:1])

                # attn (raw exp) transposed into [s%128, s//128] layout
                atps = ps_at.tile([128, NT], f32, name=f"atps_{b}", tag="atps")
                for j in range(NT):
                    nc.tensor.transpose(
                        atps[:, j:j + 1],
                        scores[0:1, j * 128:(j + 1) * 128],
                        iden[0:1, 0:1],
                    )
                attn_t = atnp.tile([128, NT], f32, name=f"attn_t_{b}",
                                   tag="attn_t")
                nc.vector.tensor_copy(attn_t[:, :], atps[:, :])

                # normalized attn output row (off the critical path, on ACT)
                nc.scalar.mul(scores[0:1, :], scores[0:1, :], rec[0:1, 0:1])
                nc.sync.dma_start(attn_d[b:b + 1, :], scores[0:1, :])

                # queue phase 2 from the resident x tiles
                for blk in range(NBLK):
                    pending_ph2.append(
                        (lambda bb, kk, xt, at, rc:
                         lambda: emit_ph2(bb, kk, xt, at, rc))
                        (b, blk, xin_tiles[blk], attn_t, rec128)
                    )

            for b in range(BC):
                scores = softp.tile([1, S], f32, name=f"scores_{b}",
                                    tag="scores")
                bmax = smallp.tile([1, NBLK], f32, name=f"bmax_{b}",
                                   tag="bmax")
                xin_tiles = []
                # ---------------- phase 1: scores for batch b ----------------
                for blk in range(NBLK):
                    s0 = b * S + blk * 512
                    xin = xinp.tile([128, 4, 512], f32, name=f"xin_{b}_{blk}",
                                    tag="xin")
                    xin_tiles.append(xin)
                    nc.sync.dma_start(
                        xin[:, :, :],
                        x_d[s0:s0 + 512, :].rearrange("(ss p) k -> p ss k", p=128),
                    )
                    xT = []
                    for kk in range(4):
                        xps = ps_xt.tile([128, 512], f32,
                                         name=f"xps_{b}_{blk}_{kk}", tag="xps")
                        for ss in range(4):
                            nc.tensor.transpose(
                                xps[:, ss * 128:(ss + 1) * 128],
                                xin[:, ss, kk * 128:(kk + 1) * 128],
                                iden[:, :],
                            )
                        xsb = xtp.tile([128, 512], f32r,
                                       name=f"xT_{b}_{blk}_{kk}", tag="xT")
                        nc.vector.tensor_copy(xsb[:, :], xps[:, :])
                        xT.append(xsb)

                    ths = []
                    for hh in range(4):
                        pre = ps_pre.tile([128, 512], f32,
                                          name=f"pre_{b}_{blk}_{hh}", tag="pre")
                        for kk in range(4):
                            nc.tensor.matmul(
                                pre[:, :],
                                w1t[:, kk * 512 + hh * 128:
                                    kk * 512 + (hh + 1) * 128],
                                xT[kk][:, :],
                                start=(kk == 0), stop=(kk == 3),
                            )
                        th = ttp.tile([128, 512], f32r,
                                      name=f"t_{b}_{blk}_{hh}", tag="t")
                        nc.scalar.activation(
                            th[:, :], pre[:, :], TANH,
                            bias=bias[:, b * 4 + hh:b * 4 + hh + 1], scale=1.0,
                        )
                        ths.append(th)

                    def emit_mv(blk=blk, ths=ths, scores=scores,
                                bmax=bmax, b=b):
                        sc = ps_sc.tile([1, 512], f32, name=f"sc_{b}_{blk}",
                                        tag="sc")
                        for hh in range(4):
                            nc.tensor.matmul(
                                sc[0:1, :],
                                vcol[:, hh:hh + 1],
                                ths[hh][:, :],
                                start=(hh == 0), stop=(hh == 3),
                            )
                        nc.scalar.copy(
                            scores[0:1, blk * 512:(blk + 1) * 512], sc[0:1, :])
                        nc.vector.reduce_max(bmax[0:1, blk:blk + 1], sc[0:1, :],
                                             axis=mybir.AxisListType.X)

                    pending_mv.append(emit_mv)
                    if len(pending_mv) > 1:
                        pending_mv.pop(0)()
                    # softmax of the previous batch, two blocks in
                    if blk == 1 and pending_soft:
                        pending_soft.pop(0)()
                    # interleave one phase-2 block of the previous batch
                    if pending_ph2:
                        pending_ph2.pop(0)()

                pending_soft.append(
                    (lambda bb, sc_, bm, xt:
                     lambda: emit_soft(bb, sc_, bm, xt))
                    (b, scores, bmax, xin_tiles)
                )

            while pending_mv:
                pending_mv.pop(0)()
            while pending_soft:
                pending_soft.pop(0)()
            while pending_ph2:
                pending_ph2.pop(0)()

    nc.compile()
    return nc


def _get_compiled():
    global _compiled
    if _compiled is None:
        _compiled = _build()
    return _compiled


def kernel(inputs, hidden_states, W_w, W_b, V_w, V_b):
    from concourse.bass_utils import run_bass_kernel_spmd

    x = np.ascontiguousarray(np.asarray(inputs, dtype=np.float32))
    h = np.asarray(hidden_states, dtype=np.float32)[0]          # (B, H)
    W_w = np.asarray(W_w, dtype=np.float32)
    W_b = np.asarray(W_b, dtype=np.float32)
    V_w = np.asarray(V_w, dtype=np.float32)
    V_b = np.asarray(V_b, dtype=np.float32)  # noqa: F841  (softmax-invariant)

    W1T = np.ascontiguousarray(W_w[:, :H].T)                     # (K, H)
    w1t_host = np.ascontiguousarray(
        W1T.reshape(4, 128, H).transpose(1, 0, 2).reshape(128, 4 * H))
    vcol = V_w.sum(axis=0, dtype=np.float64).astype(np.float32)  # (H,)
    vcol_host = np.ascontiguousarray(vcol.reshape(4, 128).T)     # (128, 4)
    bias_all = (h.astype(np.float64) @ W_w[:, H:].T.astype(np.float64)
                + W_b).astype(np.float32)                        # (B, H)
    iden = np.eye(128, dtype=np.float32)

    nc = _get_compiled()
    in_maps = []
    for c in range(N_CORES):
        bs = slice(c * BC, (c + 1) * BC)
        bias_core = np.ascontiguousarray(
            bias_all[bs].reshape(BC, 4, 128).transpose(2, 0, 1)
            .reshape(128, 4 * BC))
        in_maps.append({
            "x": np.ascontiguousarray(x[bs].reshape(BC * S, H)),
            "w1t": w1t_host,
            "vcol": vcol_host,
            "bias": bias_core,
            "iden": iden,
        })

    r = run_bass_kernel_spmd(nc, in_maps, list(range(N_CORES)))
    global LAST_RESULTS
    LAST_RESULTS = r

    res = np.concatenate(
        [r.results[c]["res"].reshape(BC, S, H) for c in range(N_CORES)], axis=0)
    attn = np.concatenate(
        [r.results[c]["attn"].reshape(BC, S, 1) for c in range(N_CORES)], axis=0)
    return res, attn


# revision 10
# speedup vs baseline: 1.2449x; 1.2449x over previous
"""Trainium2 Bass kernel for nn_Attn additive-attention scoring.

Reference computation (B=32, S=4096, H=512):
    cat    = concat([x, broadcast(h)], -1)            # (B,S,2H)
    t      = tanh(cat @ W_w^T + W_b)                  # (B,S,H)
    v      = t @ V_w^T + V_b                          # (B,S,H)
    scores = v.sum(-1, keepdims=True)                 # (B,S,1)
    attn   = softmax(scores, axis=1)
    res    = attn * x
    return res, attn

Restructuring used here (numerically equivalent to ~1e-6):
    scores[b,s] = tanh(x[b,s] @ W1^T + bias[b]) . vcol + const
where W1 = W_w[:, :H], bias[b] = h[b] @ W_w[:, H:]^T + W_b (host, tiny),
vcol = V_w.sum(0).  The constant (V_b.sum()) drops out of the softmax.

Sharding: data-parallel over batch, 4 batches per core, weights replicated.

Device dataflow per core (one batch at a time, software-pipelined):
  phase 1   stream x[b] in 1 MiB blocks into SBUF (resident, 8 MiB/batch),
            PE-transpose each block, fp32r matmul vs W1^T, tanh (+bias) on
            ACT, then a vcol matvec on PE (pipelined one block behind so PE
            never stalls on ACT) accumulates the scores row.
  softmax   on-chip over the [1, 4096] scores row (exp in place with
            accum_out); its serial chain is emitted two blocks into the
            NEXT batch's phase 1 so the PE never waits on it.
  phase 2   multiply the *resident* x blocks by raw-exp attn (PE-transposed
            to the partition dim) and 1/Z (second tensor_scalar operand)
            in place and stream res out.
  x is read from HBM exactly once: ~64 MiB DMA per core total.
"""

import sys

sys.path.insert(0, "/opt/trn_rl_repo")

import numpy as np

B, S, H = 32, 4096, 512
N_CORES = 8
BC = B // N_CORES      # batches per core
NBLK = S // 512        # 512-row blocks per batch
NT = S // 128          # 128-row tiles per batch

TRACE = False          # unused; profiling is driven externally (test.py)
LAST_RESULTS = None    # BassKernelResults of the last run

_compiled = None


def _build():
    import concourse.bacc as bacc
    import concourse.mybir as mybir
    from concourse import tile

    f32 = mybir.dt.float32
    f32r = mybir.dt.float32r
    TANH = mybir.ActivationFunctionType.Tanh
    EXP = mybir.ActivationFunctionType.Exp
    MULT = mybir.AluOpType.mult

    nc = bacc.Bacc("TRN2", target_bir_lowering=False, debug=False,
                   num_devices=N_CORES)

    x_d = nc.dram_tensor("x", [BC * S, H], f32, kind="ExternalInput").ap()
    w1t_d = nc.dram_tensor("w1t", [128, 4 * H], f32, kind="ExternalInput").ap()
    vcol_d = nc.dram_tensor("vcol", [128, 4], f32, kind="ExternalInput").ap()
    bias_d = nc.dram_tensor("bias", [128, 4 * BC], f32, kind="ExternalInput").ap()
    iden_d = nc.dram_tensor("iden", [128, 128], f32, kind="ExternalInput").ap()
    res_d = nc.dram_tensor("res", [BC * S, H], f32, kind="ExternalOutput").ap()
    attn_d = nc.dram_tensor("attn", [BC, S], f32, kind="ExternalOutput").ap()

    with tile.TileContext(nc) as tc:
        with (
            tc.tile_pool(name="const", bufs=1) as constp,
            tc.tile_pool(name="xin", bufs=14) as xinp,
            tc.tile_pool(name="xtsb", bufs=8) as xtp,
            tc.tile_pool(name="tt", bufs=8) as ttp,
            tc.tile_pool(name="soft", bufs=2) as softp,
            tc.tile_pool(name="small", bufs=4) as smallp,
            tc.tile_pool(name="atn", bufs=2) as atnp,
            tc.tile_pool(name="ps_xt", bufs=2, space="PSUM") as ps_xt,
            tc.tile_pool(name="ps_pre", bufs=2, space="PSUM") as ps_pre,
            tc.tile_pool(name="ps_sc", bufs=2, space="PSUM") as ps_sc,
            tc.tile_pool(name="ps_at", bufs=1, space="PSUM") as ps_at,
        ):
            iden = constp.tile([128, 128], f32, name="iden_sb")
            nc.sync.dma_start(iden[:, :], iden_d[:, :])
            w1t_f = xinp.tile([128, 4 * H], f32, name="w1t_f_sb", tag="xin")
            nc.sync.dma_start(w1t_f[:, :], w1t_d[:, :])
            w1t = constp.tile([128, 4 * H], f32r, name="w1t_sb")
            nc.vector.tensor_copy(w1t[:, :], w1t_f[:, :])
            vcol_f = constp.tile([128, 4], f32, name="vcol_f_sb")
            nc.sync.dma_start(vcol_f[:, :], vcol_d[:, :])
            vcol = constp.tile([128, 4], f32r, name="vcol_sb")
            nc.vector.tensor_copy(vcol[:, :], vcol_f[:, :])
            bias = constp.tile([128, 4 * BC], f32, name="bias_sb")
            nc.sync.dma_start(bias[:, :], bias_d[:, :])
            ones = constp.tile([1, 128], f32, name="ones_sb")
            nc.vector.memset(ones[0:1, :], 1.0)

            # PE warm-up: transposes of the identity (first DMA to land)
            # while the weights / first x block are still in flight, so the
            # HAM clock gate opens before real work starts.  Never read.
            warm_ps = ps_sc.tile([128, 512], f32, name="warm_ps", tag="sc")
            for _w in range(24):
                nc.tensor.transpose(warm_ps[:, 0:128], iden[:, :], iden[:, :])

            # deferred emitters for cross-block/batch software pipelining
            pending_mv = []    # score matvec of the previous block
            pending_soft = []  # softmax + attnT of the previous batch
            pending_ph2 = []   # phase-2 of the previous batch

            def emit_ph2(b, blk, xin_t, attn_t, rec128):
                s0 = b * S + blk * 512
                for ss in range(4):
                    j = blk * 4 + ss
                    nc.vector.tensor_scalar(
                        xin_t[:, ss, :], xin_t[:, ss, :],
                        attn_t[:, j:j + 1], rec128[:, 0:1],
                        op0=MULT, op1=MULT,
                    )
                nc.gpsimd.dma_start(
                    res_d[s0:s0 + 512, :].rearrange("(ss p) k -> p ss k", p=128),
                    xin_t[:, :, :],
                )

            def emit_soft(b, scores, bmax, xin_tiles):
                # softmax over S for batch b (exp in place over scores)
                mx = smallp.tile([1, 1], f32, name=f"mx_{b}", tag="mx")
                nc.vector.reduce_max(mx[0:1, 0:1], bmax[0:1, :],
                                     axis=mybir.AxisListType.X, negate=True)
                sm = smallp.tile([1, 1], f32, name=f"sm_{b}", tag="sm")
                nc.scalar.activation(scores[0:1, :], scores[0:1, :], EXP,
                                     bias=mx[0:1, 0:1], scale=1.0,
                                     accum_out=sm[0:1, 0:1])
                rec = smallp.tile([1, 1], f32, name=f"rec_{b}", tag="rec")
                nc.vector.reciprocal(rec[0:1, 0:1], sm[0:1, 0:1])

                # broadcast 1/Z to all 128 partitions: ones^T @ rec on PE
                rc_ps = ps_at.tile([128, 1], f32, name=f"rcps_{b}", tag="rcb")
                nc.tensor.matmul(rc_ps[:, 0:1], ones[0:1, :], rec[0:1, 0:1],
                                 start=True, stop=True)
                rec128 = atnp.tile([128, 1], f32, name=f"rec128_{b}", tag="rc")
                nc.vector.tensor_copy(rec128[:, 0:1], rc_ps[:, 0:1])

                # attn (raw exp) transposed into [s%128, s//128] layout
                atps = ps_at.tile([128, NT], f32, name=f"atps_{b}", tag="atps")
                for j in range(NT):
                    nc.tensor.transpose(
                        atps[:, j:j + 1],
                        scores[0:1, j * 128:(j + 1) * 128],
                        iden[0:1, 0:1],
                    )
                attn_t = atnp.tile([128, NT], f32, name=f"attn_t_{b}",
                                   tag="attn_t")
                nc.vector.tensor_copy(attn_t[:, :], atps[:, :])

                # normalized attn output row (off the critical path, on ACT)
                nc.scalar.mul(scores[0:1, :], scores[0:1, :], rec[0:1, 0:1])
                nc.gpsimd.dma_start(attn_d[b:b + 1, :], scores[0:1, :])

                # queue phase 2 from the resident x tiles
                for blk in range(NBLK):
                    pending_ph2.append(
                        (lambda bb, kk, xt, at, rc:
                         lambda: emit_ph2(bb, kk, xt, at, rc))
                        (b, blk, xin_tiles[blk], attn_t, rec128)
                    )

            for b in range(BC):
                scores = softp.tile([1, S], f32, name=f"scores_{b}",
                                    tag="scores")
                bmax = smallp.tile([1, NBLK], f32, name=f"bmax_{b}",
                                   tag="bmax")
                xin_tiles = []
                # ---------------- phase 1: scores for batch b ----------------
                for blk in range(NBLK):
                    s0 = b * S + blk * 512
                    xin = xinp.tile([128, 4, 512], f32, name=f"xin_{b}_{blk}",
                                    tag="xin")
                    xin_tiles.append(xin)
                    nc.sync.dma_start(
                        xin[:, :, :],
                        x_d[s0:s0 + 512, :].rearrange("(ss p) k -> p ss k", p=128),
                    )
                    xT = []
                    for kk in range(4):
                        xps = ps_xt.tile([128, 512], f32,
                                         name=f"xps_{b}_{blk}_{kk}", tag="xps")
                        for ss in range(4):
                            nc.tensor.transpose(
                                xps[:, ss * 128:(ss + 1) * 128],
                                xin[:, ss, kk * 128:(kk + 1) * 128],
                                iden[:, :],
                            )
                        xsb = xtp.tile([128, 512], f32r,
                                       name=f"xT_{b}_{blk}_{kk}", tag="xT")
                        nc.vector.tensor_copy(xsb[:, :], xps[:, :])
                        xT.append(xsb)

                    ths = []
                    for hh in range(4):
                        pre = ps_pre.tile([128, 512], f32,
                                          name=f"pre_{b}_{blk}_{hh}", tag="pre")
                        for kk in range(4):
                            nc.tensor.matmul(
                                pre[:, :],
                                w1t[:, kk * 512 + hh * 128:
                                    kk * 512 + (hh + 1) * 128],
                                xT[kk][:, :],
                                start=(kk == 0), stop=(kk == 3),
                            )
                        th = ttp.tile([128, 512], f32r,
                                      name=f"t_{b}_{blk}_{hh}", tag="t")
                        nc.scalar.activation(
                            th[:, :], pre[:, :], TANH,
                            bias=bias[:, b * 4 + hh:b * 4 + hh + 1], scale=1.0,
                        )
                        ths.append(th)

                    def emit_mv(blk=blk, ths=ths, scores=scores,
                                bmax=bmax, b=b):
                        sc = ps_sc.tile([1, 512], f32, name=f"sc_{b}_{blk}",
                                        tag="sc")
                        for hh in range(4):
                            nc.tensor.matmul(
                                sc[0:1, :],
                                vcol[:, hh:hh + 1],
                                ths[hh][:, :],
                                start=(hh == 0), stop=(hh == 3),
                            )
                        nc.scalar.copy(
                            scores[0:1, blk * 512:(blk + 1) * 512], sc[0:1, :])
                        nc.vector.reduce_max(bmax[0:1, blk:blk + 1], sc[0:1, :],
                                             axis=mybir.AxisListType.X)

                    pending_mv.append(emit_mv)
                    if len(pending_mv) > 1:
                        pending_mv.pop(0)()
                    # softmax of the previous batch, two blocks in
                    if blk == 1 and pending_soft:
                        pending_soft.pop(0)()
                    # interleave one phase-2 block of the previous batch
                    if pending_ph2:
                        pending_ph2.pop(0)()

                pending_soft.append(
                    (lambda bb, sc_, bm, xt:
                     lambda: emit_soft(bb, sc_, bm, xt))
                    (b, scores, bmax, xin_tiles)
                )

            while pending_mv:
                pending_mv.pop(0)()
            while pending_soft:
                pending_soft.pop(0)()
            while pending_ph2:
                pending_ph2.pop(0)()

    nc.compile()
    return nc


def _get_compiled():
    global _compiled
    if _compiled is None:
        _compiled = _build()
    return _compiled


def kernel(inputs, hidden_states, W_w, W_b, V_w, V_b):
    from concourse.bass_utils import run_bass_kernel_spmd

    x = np.ascontiguousarray(np.asarray(inputs, dtype=np.float32))
    h = np.asarray(hidden_states, dtype=np.float32)[0]          # (B, H)
    W_w = np.asarray(W_w, dtype=np.float32)
    W_b = np.asarray(W_b, dtype=np.float32)
    V_w = np.asarray(V_w, dtype=np.float32)
    V_b = np.asarray(V_b, dtype=np.float32)  # noqa: F841  (softmax-invariant)

    W1T = np.ascontiguousarray(W_w[:, :H].T)                     # (K, H)
    w1t_host = np.ascontiguousarray(
        W1T.reshape(4, 128, H).transpose(1, 0, 2).reshape(128, 4 * H))
    vcol = V_w.sum(axis=0, dtype=np.float64).astype(np.float32)  # (H,)
    vcol_host = np.ascontiguousarray(vcol.reshape(4, 128).T)     # (128, 4)
    bias_all = (h.astype(np.float64) @ W_w[:, H:].T.astype(np.float64)
                + W_b).astype(np.float32)                        # (B, H)
    iden = np.eye(128, dtype=np.float32)

    nc = _get_compiled()
    in_maps = []
    for c in range(N_CORES):
        bs = slice(c * BC, (c + 1) * BC)
        bias_core = np.ascontiguousarray(
            bias_all[bs].reshape(BC, 4, 128).transpose(2, 0, 1)
            .reshape(128, 4 * BC))
        in_maps.append({
            "x": np.ascontiguousarray(x[bs].reshape(BC * S, H)),
            "w1t": w1t_host,
            "vcol": vcol_host,
            "bias": bias_core,
            "iden": iden,
        })

    r = run_bass_kernel_spmd(nc, in_maps, list(range(N_CORES)))
    global LAST_RESULTS
    LAST_RESULTS = r

    res = np.concatenate(
        [r.results[c]["res"].reshape(BC, S, H) for c in range(N_CORES)], axis=0)
    attn = np.concatenate(
        [r.results[c]["attn"].reshape(BC, S, 1) for c in range(N_CORES)], axis=0)
    return res, attn
